# revision 50
# baseline (speedup 1.0000x reference)
"""Causal GQA self-attention (B=2, S=2048, H=2048, 16 q-heads / 4 kv-heads,
head_dim=128, RoPE) as a Bass/Tile kernel on 8 TRN2 NeuronCores.

Sharding: tensor-parallel over heads. Core c owns q-heads {2c, 2c+1} and
kv-head c//2; it computes a full [B, S, H] partial of the output projection
(o_partial = attn_out_c @ wo_c) and the host sums the 8 partials.

Layout/engine notes (v2):
 - everything IO is fp16: x is pre-transposed on the host to xT [B, H, S]
   fp16, weights fp16, output partials fp16 (host accumulates in fp64).
   fp16 matmuls run at the same PE rate as fp32r but halve HBM traffic and
   SBUF footprint, and carry a 10-bit mantissa (same as fp32r).
 - q/k are produced transposed ([head_dim, S]) straight out of the PE
   (lhsT = weight tile, rhs = xT tile). RoPE's rotate-half is a partition
   permutation done as a PE matmul against a constant signed-permutation
   matrix; the elementwise cos/sin work runs on DVE in fp16.
 - v is transposed back to natural layout with PE transposes.
 - scores are computed transposed (sT[kj, qi] = kT_j^T . qT). The causal
   mask for diagonal blocks is PRE-LOADED into PSUM by a PE matmul
   (iden^T @ mask) that opens the accumulation group, so no DVE mask add
   sits on the QK->exp critical path.
 - softmax is max-free (scores ~N(0,1)); exp runs on Act; the denominator
   is accumulated by an all-ones matmul into PSUM alongside PV.
 - the attention j-loop is software-pipelined: PV/den matmuls for block
   j-1 are emitted AFTER the QK matmuls for block j, so the PE never
   head-of-line blocks on exp(j).
 - the per-tile softmax division runs entirely on DVE off the critical
   path: reciprocal_approx_fast(den) straight from PSUM (~18 good bits),
   then one tensor_tensor multiply outT(PSUM) x r -> aT fp16.
 - per-batch homes are double-buffered so batch 1's projections overlap
   batch 0's attention.
"""

import math

import numpy as np
import ml_dtypes

import concourse.bass as bass
import concourse.tile as tile
from concourse import mybir
from concourse.bass_utils import run_bass_kernel_spmd

F32 = mybir.dt.float32
F16 = mybir.dt.float16
FP8 = mybir.dt.float8e4
AF = mybir.ActivationFunctionType
DR = mybir.MatmulPerfMode.DoubleRow

B, S, H = 2, 2048, 2048
NH, NKV, HD = 16, 4, 128
N_CORES = 8
KT = H // 128          # 16 k-tiles over the H contraction
SC = 512               # proj s-chunk width
NSC = S // SC          # 4
QT = 1024              # attention qi tile width
NQT = S // QT          # 2
SCALE = 1.0 / math.sqrt(HD)
ROPE_BASE = 10000.0
MASKVAL = -60000.0     # fp16-safe additive causal mask
EXPSCALE = SCALE


def _alu(name):
    from concourse.alu_op_type import AluOpType

    return getattr(AluOpType, name)


def legalize_waits(nc, cap=1):
    """walrus in this container accepts at most one sync-wait per
    instruction; move excess waits onto NoOp carriers just before the
    instruction on the same engine (sequencers run waits in order, so this
    is semantically identical)."""
    n_split = 0
    for f in nc.m.functions:
        for blk in f.blocks:
            if not any(
                i.sync_info is not None and len(i.sync_info.on_wait) > cap
                for i in blk.instructions
            ):
                continue
            new_insts = []
            for inst in blk.instructions:
                si = inst.sync_info
                waits = list(si.on_wait) if si is not None else []
                if len(waits) > cap:
                    for k, w in enumerate(waits[:-cap]):
                        new_insts.append(
                            mybir.InstNoOp(
                                name=f"{inst.name}-wsplit{k}",
                                engine=inst.engine,
                                sync_info=mybir.SyncInfo(on_wait=[w], on_update=[]),
                            )
                        )
                        n_split += 1
                    inst.sync_info = mybir.SyncInfo(
                        on_wait=waits[-cap:], on_update=list(si.on_update)
                    )
                new_insts.append(inst)
            blk.instructions = new_insts
    return n_split


def build_nc(legalize=True):
    mult = _alu("mult")
    add = _alu("add")

    nc = bass.Bass(trn_type="TRN2", target_bir_lowering=False)

    xT_d = nc.dram_tensor("xT", [B, H, S], F16, kind="ExternalInput")
    wq_d = nc.dram_tensor("wq", [H, 2 * HD], F16, kind="ExternalInput")
    wk_d = nc.dram_tensor("wk", [H, HD], F16, kind="ExternalInput")
    wv_d = nc.dram_tensor("wv", [H, HD], F16, kind="ExternalInput")
    wo_d = nc.dram_tensor("wo", [2 * HD, H], F16, kind="ExternalInput")
    cos_d = nc.dram_tensor("cosT", [HD, S], F16, kind="ExternalInput")
    sinrot_d = nc.dram_tensor("sinrotT", [HD, S], F16, kind="ExternalInput")
    mask_d = nc.dram_tensor("addmask", [128, 128], F16, kind="ExternalInput")
    rotm_d = nc.dram_tensor("rotmT", [128, 128], F16, kind="ExternalInput")
    iden_d = nc.dram_tensor("iden", [128, 128], F16, kind="ExternalInput")
    o_d = nc.dram_tensor("o", [B, S, H], F16, kind="ExternalOutput")

    with tile.TileContext(nc) as tc:
        with (
            tc.tile_pool(name="consts", bufs=1) as consts,
            tc.tile_pool(name="xpool", bufs=2) as xpool,
            tc.tile_pool(name="homes", bufs=2) as homes,
            tc.tile_pool(name="stage", bufs=3) as stage,
            tc.tile_pool(name="ptp", bufs=3) as ptp,
            tc.tile_pool(name="epi", bufs=2) as epi,
            tc.tile_pool(name="opool", bufs=3) as opool,
            tc.tile_pool(name="ps", bufs=4, space="PSUM") as ps,
        ):
            # ---- constants. Only wq is issued up front: the first matmul
            #      needs just wq + the first x slice; everything else is
            #      issued inside the first chunk's body so the startup DMA
            #      prefix stays minimal. ----
            wq_sb = consts.tile([128, KT, 2 * HD], F16, tag="wq_sb")
            wq_src = wq_d.ap().rearrange("(k p) d -> p k d", p=128)
            # k-tile 0-1 first so the very first matmul can start early
            nc.sync.dma_start(out=wq_sb[:, 0:2, :], in_=wq_src[:, 0:2, :])
            nc.sync.dma_start(out=wq_sb[:, 2:KT, :], in_=wq_src[:, 2:KT, :])
            wk_sb = consts.tile([128, KT, HD], F16, tag="wk_sb")
            wv_sb = consts.tile([128, KT, HD], F16, tag="wv_sb")
            cos_sb = consts.tile([HD, S], F16, tag="cos_sb")
            sinrot_sb = consts.tile([HD, S], F16, tag="sinrot_sb")
            rotm_sb = consts.tile([128, 128], F16, tag="rotm_sb")
            iden_sb = consts.tile([128, 128], F16, tag="iden_sb")
            mask_sb = consts.tile([128, 128], F16, tag="mask_sb")
            wo_sb = consts.tile([128, 2, H], F16, tag="wo_sb")
            ones_sb = consts.tile([128, 128], F16, tag="ones_sb")
            nc.vector.memset(ones_sb, 1.0)
            consts_loaded = [False]

            def load_remaining_consts():
                nc.sync.dma_start(out=wk_sb, in_=wk_d.ap().rearrange("(k p) d -> p k d", p=128))
                nc.sync.dma_start(out=wv_sb, in_=wv_d.ap().rearrange("(k p) d -> p k d", p=128))
                nc.sync.dma_start(out=cos_sb, in_=cos_d.ap())
                nc.sync.dma_start(out=sinrot_sb, in_=sinrot_d.ap())
                nc.sync.dma_start(out=rotm_sb, in_=rotm_d.ap())
                nc.sync.dma_start(out=iden_sb, in_=iden_d.ap())
                nc.sync.dma_start(out=mask_sb, in_=mask_d.ap())
                nc.sync.dma_start(out=wo_sb, in_=wo_d.ap().rearrange("(c p) n -> p c n", p=128))

            # phase-C work queue: o-projection half-tiles are emitted lazily,
            # interleaved into later phases' PE streams so the PE has fill-in
            # work while Act chews on attention exps (in-order engines need
            # emission-level interleaving)
            cqueue = []
            evac_rr = [0]

            def drain_c(n=1):
                for _ in range(min(n, len(cqueue))):
                    cqueue.pop(0)()

            def make_c_emitter(bb, aT, m, half):
                # one o-projection half-tile: 4 matmuls into PSUM, evac to
                # fp16 (DMA cannot read PSUM), DMA out. Evac alternates
                # DVE/Act so neither engine becomes the bottleneck.
                def emit():
                    ms = slice(m * 128, (m + 1) * 128)
                    pso = ps.tile([128, 1024], F32, tag="ps", name="pso")
                    for ci in range(2):
                        for n0 in range(0, 1024, 512):
                            nc.tensor.matmul(
                                pso[:, n0:n0 + 512], aT[:, ci, ms],
                                wo_sb[:, ci, half * 1024 + n0: half * 1024 + n0 + 512],
                                start=(ci == 0), stop=(ci == 1),
                            )
                    os_sb = opool.tile([128, 1024], F16, tag="os_sb", name="os_sb")
                    # evacuation is the DVE hog (~78us of 1x f32 CASTs);
                    # give every 3rd one to Act, which has ~13us/batch of
                    # slack under the exp stream in attention windows
                    evac_rr[0] += 1
                    if evac_rr[0] % 3 == 0:
                        nc.scalar.copy(os_sb, pso)
                    else:
                        nc.vector.tensor_copy(os_sb, pso)
                    nc.sync.dma_start(
                        out=o_d.ap()[bb, ms, half * 1024:(half + 1) * 1024],
                        in_=os_sb,
                    )
                return emit

            for b in range(B):
                # ---- per-batch homes (double-buffered across batches) ----
                q0_sb = homes.tile([HD, S], F16, tag="q0_sb")
                q1_sb = homes.tile([HD, S], F16, tag="q1_sb")
                kT_sb = homes.tile([HD, S], F16, tag="kT_sb")
                vp_sb = homes.tile([128, KT, HD], F16, tag="vp_sb")
                aT_sb = homes.tile([128, 2, S], F16, tag="aT_sb")
                q_homes = [q0_sb, q1_sb]

                # ================= phase A: QKV projections + RoPE ========
                for c in range(NSC):
                    cs = slice(c * SC, (c + 1) * SC)
                    xc = xpool.tile([128, KT, SC], F16, tag="xc")
                    xsrc = xT_d.ap()[b].rearrange("(k p) s -> p k s", p=128)[:, :, cs]
                    # split the chunk load so the k=0 matmuls start sooner
                    if b == 0 and c == 0:
                        nc.sync.dma_start(out=xc[:, 0:2, :], in_=xsrc[:, 0:2, :])
                        nc.sync.dma_start(out=xc[:, 2:KT // 2, :], in_=xsrc[:, 2:KT // 2, :])
                    else:
                        nc.sync.dma_start(out=xc[:, 0:KT // 2, :], in_=xsrc[:, 0:KT // 2, :])
                    nc.sync.dma_start(out=xc[:, KT // 2:KT, :], in_=xsrc[:, KT // 2:KT, :])
                    if not consts_loaded[0]:
                        load_remaining_consts()
                        consts_loaded[0] = True
                    psq = ps.tile([128, 2 * SC], F32, tag="ps")
                    pskv = ps.tile([128, 2 * SC], F32, tag="ps")
                    for k in range(KT):
                        st = dict(start=(k == 0), stop=(k == KT - 1))
                        nc.tensor.matmul(psq[:, 0:SC], wq_sb[:, k, 0:HD], xc[:, k, :], **st)
                        nc.tensor.matmul(psq[:, SC:2 * SC], wq_sb[:, k, HD:2 * HD], xc[:, k, :], **st)
                    for k in range(KT):
                        st = dict(start=(k == 0), stop=(k == KT - 1))
                        nc.tensor.matmul(pskv[:, 0:SC], wk_sb[:, k, :], xc[:, k, :], **st)
                        nc.tensor.matmul(pskv[:, SC:2 * SC], wv_sb[:, k, :], xc[:, k, :], **st)

                    # raw q/k to SBUF fp16; rotate-half via PE matmul with the
                    # signed-permutation constant; rope combine on DVE
                    raws = []
                    for i, src in enumerate((psq[:, 0:SC], psq[:, SC:2 * SC],
                                             pskv[:, 0:SC])):
                        raw = stage.tile([128, SC], F16, tag=f"raw{i}")
                        nc.scalar.copy(raw, src)
                        raws.append(raw)
                    psrot = ps.tile([128, 2 * SC], F32, tag="ps")
                    psrkv = ps.tile([128, SC], F32, tag="ps")
                    nc.tensor.matmul(psrot[:, 0:SC], rotm_sb, raws[0], start=True, stop=True)
                    nc.tensor.matmul(psrot[:, SC:2 * SC], rotm_sb, raws[1], start=True, stop=True)
                    nc.tensor.matmul(psrkv, rotm_sb, raws[2], start=True, stop=True)
                    # v: transpose to natural layout via PE (4x 128x128)
                    vt_sb = stage.tile([128, SC], F16, tag="vt_sb")
                    nc.scalar.copy(vt_sb, pskv[:, SC:2 * SC])
                    pvt = ps.tile([128, SC], F16, tag="ps")
                    for j in range(SC // 128):
                        nc.tensor.transpose(
                            pvt[:, j * 128:(j + 1) * 128],
                            vt_sb[:, j * 128:(j + 1) * 128],
                            iden_sb,
                        )
                    nc.vector.tensor_copy(
                        vp_sb[:, c * (SC // 128):(c + 1) * (SC // 128), :],
                        pvt,
                    )
                    for i, home in enumerate((q0_sb, q1_sb, kT_sb)):
                        rsrc = psrot[:, i * SC:(i + 1) * SC] if i < 2 else psrkv
                        tmp = stage.tile([128, SC], F16, tag="tmp")
                        nc.vector.tensor_tensor(tmp, rsrc, sinrot_sb[:, cs], mult)
                        nc.vector.tensor_tensor(home[:, cs], raws[i], cos_sb[:, cs], mult)
                        nc.vector.tensor_tensor(home[:, cs], home[:, cs], tmp, add)

                # ================= phase B: attention ====================
                for t in range(NQT):
                    for h in range(2):
                        qh = q_homes[h]
                        qi0 = t * QT
                        nblk = (qi0 + QT) // 128
                        outT = ps.tile([128, QT], F32, tag="ps")
                        # softmax denominator: accumulated on DVE in fp16
                        # (cheap 2-byte mode), partition-reduced by a single
                        # ones-matmul at tile end instead of per-block
                        den_acc = epi.tile([128, QT], F16, tag="den_acc")
                        seg_touchers = {}
                        for s0 in range(0, QT, SC):
                            js = [
                                j for j in range(nblk)
                                if max(j * 128 - qi0, 0) < s0 + SC
                            ]
                            seg_touchers[s0] = (js[0], js[-1])

                        def emit_pv(j, c0, pt):
                            for s0 in range(0, QT, SC):
                                a0, a1 = max(c0, s0), s0 + SC
                                if a0 >= a1:
                                    continue
                                jf, jl = seg_touchers[s0]
                                st = dict(start=(j == jf), stop=(j == jl))
                                nc.tensor.matmul(
                                    outT[:, a0:a1], vp_sb[:, j, :], pt[:, a0:a1], **st
                                )

                        pend = None
                        for j in range(nblk):
                            kj0 = j * 128
                            r = kj0 - qi0
                            c0 = max(r, 0)
                            sT = ps.tile([128, QT], F32, tag="ps")
                            diag = r >= 0
                            for s0 in range(0, QT, SC):
                                a0, a1 = max(c0, s0), s0 + SC
                                if a0 >= a1:
                                    continue
                                if diag and a0 == c0:
                                    # preload the causal mask into PSUM via
                                    # PE, then accumulate k.q on top of it
                                    nc.tensor.matmul(
                                        sT[:, a0:a0 + 128], iden_sb, mask_sb,
                                        start=True, stop=False,
                                    )
                                    nc.tensor.matmul(
                                        sT[:, a0:a0 + 128],
                                        kT_sb[:, kj0:kj0 + 128],
                                        qh[:, qi0 + a0:qi0 + a0 + 128],
                                        start=False, stop=True,
                                    )
                                    if a0 + 128 < a1:
                                        nc.tensor.matmul(
                                            sT[:, a0 + 128:a1],
                                            kT_sb[:, kj0:kj0 + 128],
                                            qh[:, qi0 + a0 + 128:qi0 + a1],
                                            start=True, stop=True,
                                        )
                                    diag = False
                                else:
                                    nc.tensor.matmul(
                                        sT[:, a0:a1],
                                        kT_sb[:, kj0:kj0 + 128],
                                        qh[:, qi0 + a0:qi0 + a1],
                                        start=True, stop=True,
                                    )
                            pt = ptp.tile([128, QT], F16, tag="pt")
                            nc.scalar.activation(
                                out=pt[:, c0:QT], in_=sT[:, c0:QT], func=AF.Exp,
                                scale=EXPSCALE,
                            )
                            # running denominator on DVE (cheap fp16 mode)
                            if j == 0:
                                nc.vector.tensor_copy(den_acc, pt)
                            else:
                                nc.vector.tensor_tensor(
                                    den_acc[:, c0:QT], den_acc[:, c0:QT],
                                    pt[:, c0:QT], add,
                                )
                            if pend is not None:
                                emit_pv(*pend)
                                drain_c(1)
                            pend = (j, c0, pt)
                        emit_pv(*pend)
                        # partition-reduce the denominator with one ones-matmul,
                        # then 1/den = exp(-ln(den)) on the Act engine (the
                        # custom-DVE fast reciprocal doesn't compile on this
                        # toolchain and plain DVE reciprocal is ~7ns/elem)
                        den_ps = ps.tile([128, QT], F32, tag="ps")
                        nc.tensor.matmul(den_ps[:, 0:512], ones_sb, den_acc[:, 0:512],
                                         start=True, stop=True)
                        nc.tensor.matmul(den_ps[:, 512:1024], ones_sb, den_acc[:, 512:1024],
                                         start=True, stop=True)
                        r_sb = epi.tile([128, QT], F32, tag="r_sb")
                        nc.scalar.activation(out=r_sb, in_=den_ps, func=AF.Ln)
                        nc.scalar.activation(out=r_sb, in_=r_sb, func=AF.Exp, scale=-1.0)
                        nc.vector.tensor_tensor(
                            aT_sb[:, h, qi0:qi0 + QT], outT, r_sb, mult
                        )
                        if h == 1:
                            # queue this token range's o-projection work;
                            # drained into later j-loops / the next batch's
                            # phase A to keep the PE saturated
                            for m in range(t * (QT // 128), (t + 1) * (QT // 128)):
                                for half in range(2):
                                    cqueue.append(make_c_emitter(b, aT_sb, m, half))

            drain_c(len(cqueue))

    if legalize:
        legalize_waits(nc)
    return nc


_NC_CACHE = None


def _get_nc():
    global _NC_CACHE
    if _NC_CACHE is None:
        _NC_CACHE = build_nc()
    return _NC_CACHE


def _host_consts():
    inv = 1.0 / (ROPE_BASE ** (np.arange(0, HD, 2, dtype=np.float32) / HD))
    t = np.arange(S, dtype=np.float32)
    freqs = np.outer(t, inv)                       # [S, HD/2]
    emb = np.concatenate([freqs, freqs], axis=-1)  # [S, HD]
    cos = np.cos(emb)
    sin = np.sin(emb)
    cosT = np.ascontiguousarray(cos.T).astype(np.float16)     # [HD, S]
    sinrotT = np.ascontiguousarray(sin.T).astype(np.float16)
    jj, ii = np.meshgrid(np.arange(128), np.arange(128), indexing="ij")
    addmask = np.where(jj <= ii, 0.0, MASKVAL).astype(np.float16)
    # rot(q)[d] = -q[d+64] (d<64), q[d-64] (d>=64); rot = R @ q and the PE
    # computes lhsT.T @ rhs, so pass R.T as the stationary operand.
    R = np.zeros((128, 128), dtype=np.float32)
    for d in range(64):
        R[d, d + 64] = -1.0
        R[d + 64, d] = 1.0
    rotmT = np.ascontiguousarray(R.T).astype(np.float16)
    iden = np.eye(128, dtype=np.float16)
    return cosT, sinrotT, addmask, rotmT, iden


def kernel(x, wq, wk, wv, wo):
    x = np.asarray(x, dtype=np.float32)
    wq = np.asarray(wq, dtype=np.float32)
    wk = np.asarray(wk, dtype=np.float32)
    wv = np.asarray(wv, dtype=np.float32)
    wo = np.asarray(wo, dtype=np.float32)

    xT = np.ascontiguousarray(x.transpose(0, 2, 1)).astype(np.float16)  # [B, H, S]
    cosT, sinrotT, addmask, rotmT, iden = _host_consts()

    in_maps = []
    for c in range(N_CORES):
        g = c // 2  # kv head
        in_maps.append({
            "xT": xT,
            "wq": np.ascontiguousarray(wq[:, 2 * c * HD:(2 * c + 2) * HD]).astype(np.float16),
            "wk": np.ascontiguousarray(wk[:, g * HD:(g + 1) * HD]).astype(np.float16),
            "wv": np.ascontiguousarray(wv[:, g * HD:(g + 1) * HD]).astype(np.float16),
            "wo": np.ascontiguousarray(wo[2 * c * HD:(2 * c + 2) * HD, :]).astype(np.float16),
            "cosT": cosT,
            "sinrotT": sinrotT,
            "addmask": addmask,
            "rotmT": rotmT,
            "iden": iden,
        })

    nc = _get_nc()
    res = run_bass_kernel_spmd(nc, in_maps, core_ids=list(range(N_CORES)))
    globals()["_LAST_RESULT"] = res
    out = np.zeros((B, S, H), dtype=np.float64)
    for r in res.results:
        out += r["o"].astype(np.float64)
    return out.astype(np.float32)


if __name__ == "__main__":
    rng = np.random.default_rng(0)
    ins = {
        "x": rng.standard_normal((B, S, H), dtype=np.float32),
        "wq": rng.standard_normal((H, NH * HD), dtype=np.float32) * 0.02,
        "wk": rng.standard_normal((H, NKV * HD), dtype=np.float32) * 0.02,
        "wv": rng.standard_normal((H, NKV * HD), dtype=np.float32) * 0.02,
        "wo": rng.standard_normal((NH * HD, H), dtype=np.float32) * 0.02,
    }
    out = kernel(**ins)
    print("out", out.shape, out.dtype, float(np.abs(out).max()))


# revision 51
# speedup vs baseline: 1.0122x; 1.0122x over previous
"""Causal GQA self-attention (B=2, S=2048, H=2048, 16 q-heads / 4 kv-heads,
head_dim=128, RoPE) as a Bass/Tile kernel on 8 TRN2 NeuronCores.

Sharding: tensor-parallel over heads. Core c owns q-heads {2c, 2c+1} and
kv-head c//2; it computes a full [B, S, H] partial of the output projection
(o_partial = attn_out_c @ wo_c) and the host sums the 8 partials.

Layout/engine notes (v2):
 - everything IO is fp16: x is pre-transposed on the host to xT [B, H, S]
   fp16, weights fp16, output partials fp16 (host accumulates in fp64).
   fp16 matmuls run at the same PE rate as fp32r but halve HBM traffic and
   SBUF footprint, and carry a 10-bit mantissa (same as fp32r).
 - q/k are produced transposed ([head_dim, S]) straight out of the PE
   (lhsT = weight tile, rhs = xT tile). RoPE's rotate-half is a partition
   permutation done as a PE matmul against a constant signed-permutation
   matrix; the elementwise cos/sin work runs on DVE in fp16.
 - v is transposed back to natural layout with PE transposes.
 - scores are computed transposed (sT[kj, qi] = kT_j^T . qT). The causal
   mask for diagonal blocks is PRE-LOADED into PSUM by a PE matmul
   (iden^T @ mask) that opens the accumulation group, so no DVE mask add
   sits on the QK->exp critical path.
 - softmax is max-free (scores ~N(0,1)); exp runs on Act; the denominator
   is accumulated by an all-ones matmul into PSUM alongside PV.
 - the attention j-loop is software-pipelined: PV/den matmuls for block
   j-1 are emitted AFTER the QK matmuls for block j, so the PE never
   head-of-line blocks on exp(j).
 - the per-tile softmax division runs entirely on DVE off the critical
   path: reciprocal_approx_fast(den) straight from PSUM (~18 good bits),
   then one tensor_tensor multiply outT(PSUM) x r -> aT fp16.
 - per-batch homes are double-buffered so batch 1's projections overlap
   batch 0's attention.
"""

import math

import numpy as np
import ml_dtypes

import concourse.bass as bass
import concourse.tile as tile
from concourse import mybir
from concourse.bass_utils import run_bass_kernel_spmd

F32 = mybir.dt.float32
F16 = mybir.dt.float16
FP8 = mybir.dt.float8e4
AF = mybir.ActivationFunctionType
DR = mybir.MatmulPerfMode.DoubleRow

B, S, H = 2, 2048, 2048
NH, NKV, HD = 16, 4, 128
N_CORES = 8
KT = H // 128          # 16 k-tiles over the H contraction
SC = 512               # proj s-chunk width
NSC = S // SC          # 4
QT = 1024              # attention qi tile width
NQT = S // QT          # 2
SCALE = 1.0 / math.sqrt(HD)
ROPE_BASE = 10000.0
MASKVAL = -60000.0     # fp16-safe additive causal mask
EXPSCALE = SCALE


def _alu(name):
    from concourse.alu_op_type import AluOpType

    return getattr(AluOpType, name)


def legalize_waits(nc, cap=1):
    """walrus in this container accepts at most one sync-wait per
    instruction; move excess waits onto NoOp carriers just before the
    instruction on the same engine (sequencers run waits in order, so this
    is semantically identical)."""
    n_split = 0
    for f in nc.m.functions:
        for blk in f.blocks:
            if not any(
                i.sync_info is not None and len(i.sync_info.on_wait) > cap
                for i in blk.instructions
            ):
                continue
            new_insts = []
            for inst in blk.instructions:
                si = inst.sync_info
                waits = list(si.on_wait) if si is not None else []
                if len(waits) > cap:
                    for k, w in enumerate(waits[:-cap]):
                        new_insts.append(
                            mybir.InstNoOp(
                                name=f"{inst.name}-wsplit{k}",
                                engine=inst.engine,
                                sync_info=mybir.SyncInfo(on_wait=[w], on_update=[]),
                            )
                        )
                        n_split += 1
                    inst.sync_info = mybir.SyncInfo(
                        on_wait=waits[-cap:], on_update=list(si.on_update)
                    )
                new_insts.append(inst)
            blk.instructions = new_insts
    return n_split


def build_nc(legalize=True):
    mult = _alu("mult")
    add = _alu("add")

    nc = bass.Bass(trn_type="TRN2", target_bir_lowering=False)

    xT_d = nc.dram_tensor("xT", [B, H, S], F16, kind="ExternalInput")
    wq_d = nc.dram_tensor("wq", [H, 2 * HD], F16, kind="ExternalInput")
    wk_d = nc.dram_tensor("wk", [H, HD], F16, kind="ExternalInput")
    wv_d = nc.dram_tensor("wv", [H, HD], F16, kind="ExternalInput")
    wo_d = nc.dram_tensor("wo", [2 * HD, H], F16, kind="ExternalInput")
    cos_d = nc.dram_tensor("cosT", [HD, S], F16, kind="ExternalInput")
    sinrot_d = nc.dram_tensor("sinrotT", [HD, S], F16, kind="ExternalInput")
    mask_d = nc.dram_tensor("addmask", [128, 128], F16, kind="ExternalInput")
    rotm_d = nc.dram_tensor("rotmT", [128, 128], F16, kind="ExternalInput")
    iden_d = nc.dram_tensor("iden", [128, 128], F16, kind="ExternalInput")
    o_d = nc.dram_tensor("o", [B, S, H], F16, kind="ExternalOutput")

    with tile.TileContext(nc) as tc:
        with (
            tc.tile_pool(name="consts", bufs=1) as consts,
            tc.tile_pool(name="xpool", bufs=2) as xpool,
            tc.tile_pool(name="homes", bufs=2) as homes,
            tc.tile_pool(name="stage", bufs=3) as stage,
            tc.tile_pool(name="ptp", bufs=3) as ptp,
            tc.tile_pool(name="epi", bufs=2) as epi,
            tc.tile_pool(name="opool", bufs=3) as opool,
            tc.tile_pool(name="ps", bufs=4, space="PSUM") as ps,
        ):
            # ---- constants. Only wq is issued up front: the first matmul
            #      needs just wq + the first x slice; everything else is
            #      issued inside the first chunk's body so the startup DMA
            #      prefix stays minimal. ----
            wq_sb = consts.tile([128, KT, 2 * HD], F16, tag="wq_sb")
            wq_src = wq_d.ap().rearrange("(k p) d -> p k d", p=128)
            # k-tile 0-1 first so the very first matmul can start early
            nc.sync.dma_start(out=wq_sb[:, 0:2, :], in_=wq_src[:, 0:2, :])
            nc.sync.dma_start(out=wq_sb[:, 2:KT, :], in_=wq_src[:, 2:KT, :])
            wk_sb = consts.tile([128, KT, HD], F16, tag="wk_sb")
            wv_sb = consts.tile([128, KT, HD], F16, tag="wv_sb")
            cos_sb = consts.tile([HD, S], F16, tag="cos_sb")
            sinrot_sb = consts.tile([HD, S], F16, tag="sinrot_sb")
            rotm_sb = consts.tile([128, 128], F16, tag="rotm_sb")
            iden_sb = consts.tile([128, 128], F16, tag="iden_sb")
            mask_sb = consts.tile([128, 128], F16, tag="mask_sb")
            wo_sb = consts.tile([128, 2, H], F16, tag="wo_sb")
            ones_sb = consts.tile([128, 128], F16, tag="ones_sb")
            nc.vector.memset(ones_sb, 1.0)
            consts_loaded = [False]

            def load_remaining_consts():
                nc.sync.dma_start(out=wk_sb, in_=wk_d.ap().rearrange("(k p) d -> p k d", p=128))
                nc.sync.dma_start(out=wv_sb, in_=wv_d.ap().rearrange("(k p) d -> p k d", p=128))
                nc.sync.dma_start(out=cos_sb, in_=cos_d.ap())
                nc.sync.dma_start(out=sinrot_sb, in_=sinrot_d.ap())
                nc.sync.dma_start(out=rotm_sb, in_=rotm_d.ap())
                nc.sync.dma_start(out=iden_sb, in_=iden_d.ap())
                nc.sync.dma_start(out=mask_sb, in_=mask_d.ap())
                nc.sync.dma_start(out=wo_sb, in_=wo_d.ap().rearrange("(c p) n -> p c n", p=128))

            # phase-C work queue: o-projection half-tiles are emitted lazily,
            # interleaved into later phases' PE streams so the PE has fill-in
            # work while Act chews on attention exps (in-order engines need
            # emission-level interleaving)
            cqueue = []
            evac_rr = [0]

            def drain_c(n=1):
                for _ in range(min(n, len(cqueue))):
                    cqueue.pop(0)()

            def make_c_emitter(bb, aT, m, half):
                # one o-projection half-tile: 4 matmuls into PSUM, evac to
                # fp16 (DMA cannot read PSUM), DMA out. Evac alternates
                # DVE/Act so neither engine becomes the bottleneck.
                def emit():
                    ms = slice(m * 128, (m + 1) * 128)
                    pso = ps.tile([128, 1024], F32, tag="ps", name="pso")
                    for ci in range(2):
                        for n0 in range(0, 1024, 512):
                            nc.tensor.matmul(
                                pso[:, n0:n0 + 512], aT[:, ci, ms],
                                wo_sb[:, ci, half * 1024 + n0: half * 1024 + n0 + 512],
                                start=(ci == 0), stop=(ci == 1),
                            )
                    os_sb = opool.tile([128, 1024], F16, tag="os_sb", name="os_sb")
                    # evacuation is the DVE hog (~78us of 1x f32 CASTs);
                    # give every 3rd one to Act, which has ~13us/batch of
                    # slack under the exp stream in attention windows
                    evac_rr[0] += 1
                    if evac_rr[0] % 3 == 0:
                        nc.scalar.copy(os_sb, pso)
                    else:
                        nc.vector.tensor_copy(os_sb, pso)
                    nc.sync.dma_start(
                        out=o_d.ap()[bb, ms, half * 1024:(half + 1) * 1024],
                        in_=os_sb,
                    )
                return emit

            for b in range(B):
                # ---- per-batch homes (double-buffered across batches) ----
                q0_sb = homes.tile([HD, S], F16, tag="q0_sb")
                q1_sb = homes.tile([HD, S], F16, tag="q1_sb")
                kT_sb = homes.tile([HD, S], F16, tag="kT_sb")
                vp_sb = homes.tile([128, KT, HD], F16, tag="vp_sb")
                aT_sb = homes.tile([128, 2, S], F16, tag="aT_sb")
                q_homes = [q0_sb, q1_sb]

                # ================= phase A: QKV projections + RoPE ========
                def do_chunk(c):
                    cs = slice(c * SC, (c + 1) * SC)
                    xc = xpool.tile([128, KT, SC], F16, tag="xc")
                    xsrc = xT_d.ap()[b].rearrange("(k p) s -> p k s", p=128)[:, :, cs]
                    # split the chunk load so the k=0 matmuls start sooner
                    if b == 0 and c == 0:
                        nc.sync.dma_start(out=xc[:, 0:2, :], in_=xsrc[:, 0:2, :])
                        nc.sync.dma_start(out=xc[:, 2:KT // 2, :], in_=xsrc[:, 2:KT // 2, :])
                    else:
                        nc.sync.dma_start(out=xc[:, 0:KT // 2, :], in_=xsrc[:, 0:KT // 2, :])
                    nc.sync.dma_start(out=xc[:, KT // 2:KT, :], in_=xsrc[:, KT // 2:KT, :])
                    if not consts_loaded[0]:
                        load_remaining_consts()
                        consts_loaded[0] = True
                    psq = ps.tile([128, 2 * SC], F32, tag="ps")
                    pskv = ps.tile([128, 2 * SC], F32, tag="ps")
                    for k in range(KT):
                        st = dict(start=(k == 0), stop=(k == KT - 1))
                        nc.tensor.matmul(psq[:, 0:SC], wq_sb[:, k, 0:HD], xc[:, k, :], **st)
                        nc.tensor.matmul(psq[:, SC:2 * SC], wq_sb[:, k, HD:2 * HD], xc[:, k, :], **st)
                    for k in range(KT):
                        st = dict(start=(k == 0), stop=(k == KT - 1))
                        nc.tensor.matmul(pskv[:, 0:SC], wk_sb[:, k, :], xc[:, k, :], **st)
                        nc.tensor.matmul(pskv[:, SC:2 * SC], wv_sb[:, k, :], xc[:, k, :], **st)

                    # raw q/k to SBUF fp16; rotate-half via PE matmul with the
                    # signed-permutation constant; rope combine on DVE
                    raws = []
                    for i, src in enumerate((psq[:, 0:SC], psq[:, SC:2 * SC],
                                             pskv[:, 0:SC])):
                        raw = stage.tile([128, SC], F16, tag=f"raw{i}")
                        nc.scalar.copy(raw, src)
                        raws.append(raw)
                    psrot = ps.tile([128, 2 * SC], F32, tag="ps")
                    psrkv = ps.tile([128, SC], F32, tag="ps")
                    nc.tensor.matmul(psrot[:, 0:SC], rotm_sb, raws[0], start=True, stop=True)
                    nc.tensor.matmul(psrot[:, SC:2 * SC], rotm_sb, raws[1], start=True, stop=True)
                    nc.tensor.matmul(psrkv, rotm_sb, raws[2], start=True, stop=True)
                    # v: transpose to natural layout via PE (4x 128x128)
                    vt_sb = stage.tile([128, SC], F16, tag="vt_sb")
                    nc.scalar.copy(vt_sb, pskv[:, SC:2 * SC])
                    pvt = ps.tile([128, SC], F16, tag="ps")
                    for j in range(SC // 128):
                        nc.tensor.transpose(
                            pvt[:, j * 128:(j + 1) * 128],
                            vt_sb[:, j * 128:(j + 1) * 128],
                            iden_sb,
                        )
                    nc.vector.tensor_copy(
                        vp_sb[:, c * (SC // 128):(c + 1) * (SC // 128), :],
                        pvt,
                    )
                    for i, home in enumerate((q0_sb, q1_sb, kT_sb)):
                        rsrc = psrot[:, i * SC:(i + 1) * SC] if i < 2 else psrkv
                        tmp = stage.tile([128, SC], F16, tag="tmp")
                        nc.vector.tensor_tensor(tmp, rsrc, sinrot_sb[:, cs], mult)
                        nc.vector.tensor_tensor(home[:, cs], raws[i], cos_sb[:, cs], mult)
                        nc.vector.tensor_tensor(home[:, cs], home[:, cs], tmp, add)

                # ================= phase B: attention ====================
                def do_btile(t, h):
                    if True:
                        qh = q_homes[h]
                        qi0 = t * QT
                        nblk = (qi0 + QT) // 128
                        outT = ps.tile([128, QT], F32, tag="ps")
                        # softmax denominator: accumulated on DVE in fp16
                        # (cheap 2-byte mode), partition-reduced by a single
                        # ones-matmul at tile end instead of per-block
                        den_acc = epi.tile([128, QT], F16, tag="den_acc")
                        seg_touchers = {}
                        for s0 in range(0, QT, SC):
                            js = [
                                j for j in range(nblk)
                                if max(j * 128 - qi0, 0) < s0 + SC
                            ]
                            seg_touchers[s0] = (js[0], js[-1])

                        def emit_pv(j, c0, pt):
                            for s0 in range(0, QT, SC):
                                a0, a1 = max(c0, s0), s0 + SC
                                if a0 >= a1:
                                    continue
                                jf, jl = seg_touchers[s0]
                                st = dict(start=(j == jf), stop=(j == jl))
                                nc.tensor.matmul(
                                    outT[:, a0:a1], vp_sb[:, j, :], pt[:, a0:a1], **st
                                )

                        pend = None
                        for j in range(nblk):
                            kj0 = j * 128
                            r = kj0 - qi0
                            c0 = max(r, 0)
                            sT = ps.tile([128, QT], F32, tag="ps")
                            diag = r >= 0
                            for s0 in range(0, QT, SC):
                                a0, a1 = max(c0, s0), s0 + SC
                                if a0 >= a1:
                                    continue
                                if diag and a0 == c0:
                                    # preload the causal mask into PSUM via
                                    # PE, then accumulate k.q on top of it
                                    nc.tensor.matmul(
                                        sT[:, a0:a0 + 128], iden_sb, mask_sb,
                                        start=True, stop=False,
                                    )
                                    nc.tensor.matmul(
                                        sT[:, a0:a0 + 128],
                                        kT_sb[:, kj0:kj0 + 128],
                                        qh[:, qi0 + a0:qi0 + a0 + 128],
                                        start=False, stop=True,
                                    )
                                    if a0 + 128 < a1:
                                        nc.tensor.matmul(
                                            sT[:, a0 + 128:a1],
                                            kT_sb[:, kj0:kj0 + 128],
                                            qh[:, qi0 + a0 + 128:qi0 + a1],
                                            start=True, stop=True,
                                        )
                                    diag = False
                                else:
                                    nc.tensor.matmul(
                                        sT[:, a0:a1],
                                        kT_sb[:, kj0:kj0 + 128],
                                        qh[:, qi0 + a0:qi0 + a1],
                                        start=True, stop=True,
                                    )
                            pt = ptp.tile([128, QT], F16, tag="pt")
                            nc.scalar.activation(
                                out=pt[:, c0:QT], in_=sT[:, c0:QT], func=AF.Exp,
                                scale=EXPSCALE,
                            )
                            # running denominator on DVE (cheap fp16 mode)
                            if j == 0:
                                nc.vector.tensor_copy(den_acc, pt)
                            else:
                                nc.vector.tensor_tensor(
                                    den_acc[:, c0:QT], den_acc[:, c0:QT],
                                    pt[:, c0:QT], add,
                                )
                            if pend is not None:
                                emit_pv(*pend)
                                drain_c(1)
                            pend = (j, c0, pt)
                        emit_pv(*pend)
                        # partition-reduce the denominator with one ones-matmul,
                        # then 1/den = exp(-ln(den)) on the Act engine (the
                        # custom-DVE fast reciprocal doesn't compile on this
                        # toolchain and plain DVE reciprocal is ~7ns/elem)
                        den_ps = ps.tile([128, QT], F32, tag="ps")
                        nc.tensor.matmul(den_ps[:, 0:512], ones_sb, den_acc[:, 0:512],
                                         start=True, stop=True)
                        nc.tensor.matmul(den_ps[:, 512:1024], ones_sb, den_acc[:, 512:1024],
                                         start=True, stop=True)
                        r_sb = epi.tile([128, QT], F32, tag="r_sb")
                        nc.scalar.activation(out=r_sb, in_=den_ps, func=AF.Ln)
                        nc.scalar.activation(out=r_sb, in_=r_sb, func=AF.Exp, scale=-1.0)
                        nc.vector.tensor_tensor(
                            aT_sb[:, h, qi0:qi0 + QT], outT, r_sb, mult
                        )
                        if h == 1:
                            # queue this token range's o-projection work;
                            # drained into later j-loops / the next batch's
                            # phase A to keep the PE saturated
                            for m in range(t * (QT // 128), (t + 1) * (QT // 128)):
                                for half in range(2):
                                    cqueue.append(make_c_emitter(b, aT_sb, m, half))

                # B(t0) only needs x-chunks 0-1; emitting it between the
                # chunk pairs hides its exp/den work under chunks 2-3's
                # PE-dense stream
                for c in (0, 1):
                    do_chunk(c)
                for h in (0, 1):
                    do_btile(0, h)
                for c in (2, 3):
                    do_chunk(c)
                for h in (0, 1):
                    do_btile(1, h)

            drain_c(len(cqueue))

    if legalize:
        legalize_waits(nc)
    return nc


_NC_CACHE = None


def _get_nc():
    global _NC_CACHE
    if _NC_CACHE is None:
        _NC_CACHE = build_nc()
    return _NC_CACHE


def _host_consts():
    inv = 1.0 / (ROPE_BASE ** (np.arange(0, HD, 2, dtype=np.float32) / HD))
    t = np.arange(S, dtype=np.float32)
    freqs = np.outer(t, inv)                       # [S, HD/2]
    emb = np.concatenate([freqs, freqs], axis=-1)  # [S, HD]
    cos = np.cos(emb)
    sin = np.sin(emb)
    cosT = np.ascontiguousarray(cos.T).astype(np.float16)     # [HD, S]
    sinrotT = np.ascontiguousarray(sin.T).astype(np.float16)
    jj, ii = np.meshgrid(np.arange(128), np.arange(128), indexing="ij")
    addmask = np.where(jj <= ii, 0.0, MASKVAL).astype(np.float16)
    # rot(q)[d] = -q[d+64] (d<64), q[d-64] (d>=64); rot = R @ q and the PE
    # computes lhsT.T @ rhs, so pass R.T as the stationary operand.
    R = np.zeros((128, 128), dtype=np.float32)
    for d in range(64):
        R[d, d + 64] = -1.0
        R[d + 64, d] = 1.0
    rotmT = np.ascontiguousarray(R.T).astype(np.float16)
    iden = np.eye(128, dtype=np.float16)
    return cosT, sinrotT, addmask, rotmT, iden


def kernel(x, wq, wk, wv, wo):
    x = np.asarray(x, dtype=np.float32)
    wq = np.asarray(wq, dtype=np.float32)
    wk = np.asarray(wk, dtype=np.float32)
    wv = np.asarray(wv, dtype=np.float32)
    wo = np.asarray(wo, dtype=np.float32)

    xT = np.ascontiguousarray(x.transpose(0, 2, 1)).astype(np.float16)  # [B, H, S]
    cosT, sinrotT, addmask, rotmT, iden = _host_consts()

    in_maps = []
    for c in range(N_CORES):
        g = c // 2  # kv head
        in_maps.append({
            "xT": xT,
            "wq": np.ascontiguousarray(wq[:, 2 * c * HD:(2 * c + 2) * HD]).astype(np.float16),
            "wk": np.ascontiguousarray(wk[:, g * HD:(g + 1) * HD]).astype(np.float16),
            "wv": np.ascontiguousarray(wv[:, g * HD:(g + 1) * HD]).astype(np.float16),
            "wo": np.ascontiguousarray(wo[2 * c * HD:(2 * c + 2) * HD, :]).astype(np.float16),
            "cosT": cosT,
            "sinrotT": sinrotT,
            "addmask": addmask,
            "rotmT": rotmT,
            "iden": iden,
        })

    nc = _get_nc()
    res = run_bass_kernel_spmd(nc, in_maps, core_ids=list(range(N_CORES)))
    globals()["_LAST_RESULT"] = res
    out = np.zeros((B, S, H), dtype=np.float64)
    for r in res.results:
        out += r["o"].astype(np.float64)
    return out.astype(np.float32)


if __name__ == "__main__":
    rng = np.random.default_rng(0)
    ins = {
        "x": rng.standard_normal((B, S, H), dtype=np.float32),
        "wq": rng.standard_normal((H, NH * HD), dtype=np.float32) * 0.02,
        "wk": rng.standard_normal((H, NKV * HD), dtype=np.float32) * 0.02,
        "wv": rng.standard_normal((H, NKV * HD), dtype=np.float32) * 0.02,
        "wo": rng.standard_normal((NH * HD, H), dtype=np.float32) * 0.02,
    }
    out = kernel(**ins)
    print("out", out.shape, out.dtype, float(np.abs(out).max()))
